# revision 20
# baseline (speedup 1.0000x reference)
"""Trainium2 Bass kernel for a 2-layer GAT (PyG semantics, eval mode).

SPMD over 8 NeuronCores, dst-sorted edge partitioning (no all-reduce):
 - conv aggregation + softmax denominators fused into one-hot matmuls
   (psum += eq.T @ [h*ex | ex]) per 128-edge tile.
 - All per-edge gathers via the batched Q7 dma_gather (one instruction per
   a few thousand rows; SWDGE fixed cost amortized).  Tables are fp16 rows
   padded to 256B/512B; src index lists are split at row 32768 per the int16
   index constraint, dst indices are device-relative.
 - Phase 0 (x @ W1ext) is computed fully redundantly on every core (cheap on
   PE) so conv1 needs no collective; conv2's h2 table is all-gathered (small).
 - Segment softmax without max subtraction (exp stays in fp32 range; softmax
   is shift-invariant, validated vs reference).
"""
import sys

sys.path.insert(0, "/opt/trn_rl_repo")

import numpy as np

import concourse.bacc as bacc
import concourse.bass as bass
import concourse.mybir as mybir
import concourse.tile as tile
from concourse.bass_utils import run_bass_kernel_spmd

P = 128
N_DEV = 8
N_NODES = 50000
F_IN, HID, N_CLS, HEADS = 128, 16, 40, 8
NEG_SLOPE = 0.2
HALF = 32768                 # int16 index limit for dma_gather

N_TILES = 392
N_PAD = N_TILES * P          # 50176
TPD = N_TILES // N_DEV       # 49
NPD = TPD * P                # 6272

C1 = HEADS * HID             # 128
D1 = C1 + HEADS              # 136 [h | a_s]
R1 = 256                     # conv1 table row (fp16): [h(128)|a_s(8)|a_d(8)|pad]
R2 = 128                     # conv2 table row (fp16): [h2lin(40)|a_s2(40:41)|a_d2(41:42)|pad]
GB = 2                       # node tiles per gather call

fp32 = mybir.dt.float32
fp16 = mybir.dt.float16
i16 = mybir.dt.int16

_CACHE = {}


def _wrap(idx_list):
    """int16 index list -> [128, n/16] wrapped layout (j at [j%16, j//16],
    replicated across the 8 16-partition groups)."""
    n = len(idx_list)
    assert n % 16 == 0
    w = np.asarray(idx_list, dtype=np.int16).reshape(n // 16, 16).T
    return np.tile(w, (8, 1))


def _preprocess(edge_index):
    src = np.concatenate([edge_index[0], np.arange(N_NODES, dtype=np.int32)])
    dst = np.concatenate([edge_index[1], np.arange(N_NODES, dtype=np.int32)])
    order = np.argsort(dst, kind="stable")
    src_s, dst_s = src[order], dst[order]
    bounds = np.searchsorted(dst_s, np.arange(N_TILES + 1) * P).astype(np.int64)

    # per node tile: split by src half, padded tile counts
    lo_lists, hi_lists = [], []
    t1_max = t2_max = 0
    for j in range(N_TILES):
        s = src_s[bounds[j]:bounds[j + 1]]
        d = dst_s[bounds[j]:bounds[j + 1]]
        lo_m = s < HALF
        lo_lists.append((s[lo_m], d[lo_m]))
        hi_lists.append((s[~lo_m], d[~lo_m]))
        t1_max = max(t1_max, (len(lo_lists[-1][0]) + P - 1) // P)
        t2_max = max(t2_max, (len(hi_lists[-1][0]) + P - 1) // P)
    T1, T2 = max(t1_max, 1), max(t2_max, 1)
    T = T1 + T2
    n_et = TPD * T

    groups = [1] * TPD
    CH = 8                     # edge tiles per gather call (<=1024 idxs)

    # per-device host arrays
    dev_widx = []   # [128, W] int16 wrapped indices (lo blocks, hi blocks, dst blocks)
    dev_dstl = []   # [128, n_et] fp16
    offs_lo, offs_hi, offs_d = [], [], []   # column offsets per group (shared)
    for d in range(N_DEV):
        cols_lo, cols_hi, cols_d = [], [], []
        dstl = np.full((n_et, P), 300, dtype=np.int32)
        jj = 0
        for g, B in enumerate(groups):
            llo, lhi, ld = [], [], []
            for k in range(B):
                j = d * TPD + jj + k
                et0 = (jj + k) * T
                for half_i, (lists, Tn, base_t) in enumerate(
                        [(lo_lists, T1, 0), (hi_lists, T2, T1)]):
                    s_h, d_h = lists[j]
                    pad = Tn * P - len(s_h)
                    idxs = np.concatenate([
                        s_h - (HALF if half_i else 0),
                        np.zeros(pad, np.int32)])
                    (llo if half_i == 0 else lhi).append(idxs)
                    dl = np.full(Tn * P, 300, np.int32)
                    dl[:len(d_h)] = d_h - j * P
                    dstl[et0 + base_t:et0 + base_t + Tn] = dl.reshape(Tn, P)
                    dd = np.zeros(Tn * P, np.int32)
                    dd[:len(d_h)] = d_h - d * NPD
                    ld.append(dd)
            llo = np.concatenate(llo)
            lhi = np.concatenate(lhi)
            ld = np.concatenate(ld)
            jj += B
            if d == 0:
                offs_lo.append(len(llo)); offs_hi.append(len(lhi)); offs_d.append(len(ld))
            if g == 0:
                blk_lo, blk_hi, blk_d = [], [], []
            # wrap each <=CH*128-idx chunk separately (one gather call each)
            for arr, blk in ((llo, blk_lo), (lhi, blk_hi), (ld, blk_d)):
                for c0 in range(0, len(arr), CH * P):
                    blk.append(_wrap(arr[c0:c0 + CH * P]))
        widx = np.concatenate(blk_lo + blk_hi + blk_d, axis=1).astype(np.int16)
        dev_widx.append(np.ascontiguousarray(widx))
        dev_dstl.append(np.ascontiguousarray(dstl.T.astype(np.float32)))
    cfg = dict(T1=T1, T2=T2, T=T, n_et=n_et, groups=groups,
               w_lo=[n // 16 for n in offs_lo],
               w_hi=[n // 16 for n in offs_hi],
               w_d=[n // 16 for n in offs_d])
    return dev_widx, dev_dstl, cfg


def _build(cfg):
    T1, T2, T = cfg["T1"], cfg["T2"], cfg["T"]
    n_et, groups = cfg["n_et"], cfg["groups"]
    w_lo, w_hi, w_d = cfg["w_lo"], cfg["w_hi"], cfg["w_d"]
    W_lo, W_hi, W_d = sum(w_lo), sum(w_hi), sum(w_d)
    W_tot = W_lo + W_hi + W_d

    nc = bacc.Bacc("TRN2", target_bir_lowering=False, debug=False,
                   num_devices=N_DEV)

    xT = nc.dram_tensor("xT", [P, N_PAD], fp16, kind="ExternalInput")
    w1ext = nc.dram_tensor("w1ext", [P, D1 + HEADS], fp16, kind="ExternalInput")
    w2ext = nc.dram_tensor("w2ext", [HID, N_CLS + 2], fp16, kind="ExternalInput")
    b1b = nc.dram_tensor("b1b", [P, HID], fp32, kind="ExternalInput")
    b2b = nc.dram_tensor("b2b", [P, N_CLS], fp32, kind="ExternalInput")
    ident = nc.dram_tensor("ident", [P, P], fp32, kind="ExternalInput")
    iota = nc.dram_tensor("iota", [P, P], fp16, kind="ExternalInput")
    widx = nc.dram_tensor("widx", [P, W_tot], i16, kind="ExternalInput")
    dstli = nc.dram_tensor("dstli", [P, n_et], fp32, kind="ExternalInput")
    xT_loc = nc.dram_tensor("xT_loc", [P, NPD], fp16, kind="ExternalInput")
    out = nc.dram_tensor("out", [NPD, N_CLS], fp32, kind="ExternalOutput")

    h_lo_t = nc.dram_tensor("h_lo_t", [HALF, R1], fp16)
    h_hi_t = nc.dram_tensor("h_hi_t", [N_PAD - HALF, R1], fp16)
    h2_ext = nc.dram_tensor("h2_ext", [N_PAD, R2], fp16)
    CMP = 44   # compact h2 row for the collective (42 used, 44 for alignment)
    h2_cmp_sh = nc.dram_tensor("h2_cmp_sh", [N_PAD, CMP], fp16, addr_space="Shared")
    h2_cmp_loc = nc.dram_tensor("h2_cmp_loc", [NPD, CMP], fp16)

    rg = [list(range(N_DEV))]
    # Device-local a_d window: the a_d gathers use device-relative dst indices
    # into the device's own node window; recomputed from xT_loc (input slice).
    ad_win = nc.dram_tensor("ad_win", [NPD, R1], fp16)
    h2_loc = nc.dram_tensor("h2_loc", [NPD, R2], fp16)

    with tile.TileContext(nc) as tc:
        with (
            tc.tile_pool(name="const", bufs=1) as cpool,
            tc.tile_pool(name="gath", bufs=3) as gp,
            tc.tile_pool(name="work", bufs=3) as wp,
            tc.tile_pool(name="small", bufs=2) as sm,
            tc.tile_pool(name="psum", bufs=2, space="PSUM") as ps,
            tc.tile_pool(name="psum2", bufs=2, space="PSUM") as ps2p,
            tc.tile_pool(name="dram", bufs=1, space="DRAM") as dr,
        ):
            # ---------------- constants
            w1_sb = cpool.tile([P, D1 + HEADS], fp16)
            nc.sync.dma_start(out=w1_sb[:], in_=w1ext[:])
            w2_sb = cpool.tile([HID, N_CLS + 2], fp16)
            nc.sync.dma_start(out=w2_sb[:], in_=w2ext[:])
            b1_sb = cpool.tile([P, HID], fp32)
            nc.sync.dma_start(out=b1_sb[:], in_=b1b[:])
            b2_sb = cpool.tile([P, N_CLS], fp32)
            nc.sync.dma_start(out=b2_sb[:], in_=b2b[:])
            id_sb = cpool.tile([P, P], fp32)
            nc.sync.dma_start(out=id_sb[:], in_=ident[:])
            iota_sb = cpool.tile([P, P], fp16)
            nc.sync.dma_start(out=iota_sb[:], in_=iota[:])
            widx_sb = cpool.tile([P, W_tot], i16)
            nc.sync.dma_start(out=widx_sb[:], in_=widx[:])
            dstl_sb = cpool.tile([P, n_et], fp32)
            nc.sync.dma_start(out=dstl_sb[:], in_=dstli[:])
            ad2c = cpool.tile([P, TPD * T], fp16)


            # ---------------- phase 0: full-redundant h_ext = x @ W1ext
            NCHUNK = 8
            for c in range(N_TILES // NCHUNK):
                xc = wp.tile([P, NCHUNK * P], fp16, tag="xc")
                nc.sync.dma_start(out=xc[:], in_=xT[:, c * NCHUNK * P:(c + 1) * NCHUNK * P])
                hst = wp.tile([P, NCHUNK * R1], fp16, tag="hst")
                for k in range(NCHUNK):
                    psh = ps.tile([P, D1 + HEADS], fp32, tag="acc", space="PSUM")
                    nc.tensor.matmul(out=psh[:], lhsT=xc[:, k * P:(k + 1) * P],
                                     rhs=w1_sb[:], start=True, stop=True)
                    nc.scalar.copy(out=hst[:, k * R1:k * R1 + D1 + HEADS], in_=psh[:])
                r0 = c * NCHUNK * P
                tgt = (h_lo_t[r0:r0 + NCHUNK * P, :] if r0 < HALF
                       else h_hi_t[r0 - HALF:r0 - HALF + NCHUNK * P, :])
                nc.sync.dma_start(
                    out=tgt.rearrange("(k p) d -> p k d", p=P),
                    in_=hst[:].rearrange("p (k d) -> p k d", d=R1))
            # device-local a_d window: recompute local rows from xT_loc
            NC2 = 7
            for c in range(TPD // NC2):
                xc2 = wp.tile([P, NC2 * P], fp16, tag="xc")
                nc.sync.dma_start(out=xc2[:], in_=xT_loc[:, c * NC2 * P:(c + 1) * NC2 * P])
                hst2 = wp.tile([P, NC2 * R1], fp16, tag="hst")
                for k in range(NC2):
                    psh = ps.tile([P, D1 + HEADS], fp32, tag="acc", space="PSUM")
                    nc.tensor.matmul(out=psh[:], lhsT=xc2[:, k * P:(k + 1) * P],
                                     rhs=w1_sb[:], start=True, stop=True)
                    nc.scalar.copy(out=hst2[:, k * R1:k * R1 + D1 + HEADS], in_=psh[:])
                nc.sync.dma_start(
                    out=ad_win[c * NC2 * P:(c + 1) * NC2 * P, :]
                        .rearrange("(k p) d -> p k d", p=P),
                    in_=hst2[:].rearrange("p (k d) -> p k d", d=R1))

            # ---------------- conv1 main
            et_base = 0
            col_lo = col_hi = col_d = 0
            jj_global = 0
            CH = 8
            for g, B in enumerate(groups):
                hg_lo = gp.tile([P, T1 * R1], fp16, tag="hg_lo")
                for c0 in range(0, T1, CH):
                    n_t = min(CH, T1 - c0); ni = n_t * P
                    nc.gpsimd.dma_gather(
                        out_ap=hg_lo[:].rearrange("p (t d) -> p t d", d=R1)[:, c0:c0 + n_t, :],
                        in_ap=h_lo_t[:, :],
                        idxs_ap=widx_sb[:, col_lo + c0 * 8:col_lo + (c0 + n_t) * 8],
                        num_idxs=ni, num_idxs_reg=ni, elem_size=R1)
                hg_hi = gp.tile([P, T2 * R1], fp16, tag="hg_hi")
                for c0 in range(0, T2, CH):
                    n_t = min(CH, T2 - c0); ni = n_t * P
                    nc.gpsimd.dma_gather(
                        out_ap=hg_hi[:].rearrange("p (t d) -> p t d", d=R1)[:, c0:c0 + n_t, :],
                        in_ap=h_hi_t[:, :],
                        idxs_ap=widx_sb[:, W_lo + col_hi + c0 * 8:W_lo + col_hi + (c0 + n_t) * 8],
                        num_idxs=ni, num_idxs_reg=ni, elem_size=R1)
                adg = gp.tile([P, T * P], fp16, tag="adg")
                for c0 in range(0, T, CH):
                    n_t = min(CH, T - c0); ni = n_t * P
                    nc.gpsimd.dma_gather(
                        out_ap=adg[:].rearrange("p (t d) -> p t d", d=P)[:, c0:c0 + n_t, :],
                        in_ap=ad_win[:, C1:R1],
                        idxs_ap=widx_sb[:, W_lo + W_hi + col_d + c0 * 8:W_lo + W_hi + col_d + (c0 + n_t) * 8],
                        num_idxs=ni, num_idxs_reg=ni, elem_size=P, elem_step=R1)
                col_lo += w_lo[g]; col_hi += w_hi[g]; col_d += w_d[g]

                for k in range(B):
                    jj = jj_global + k
                    hgl_v = hg_lo[:].rearrange("p (t d) -> p t d", d=R1)[:, k * T1:(k + 1) * T1, :]
                    hgh_v = hg_hi[:].rearrange("p (t d) -> p t d", d=R1)[:, k * T2:(k + 1) * T2, :]
                    adg_v = adg[:].rearrange("p (t d) -> p t d", d=P)[:, k * T:(k + 1) * T, 8:16]
                    # e = a_s + a_d  (batched, fp32)
                    e_sb = sm.tile([P, T * HEADS], fp32, tag="e_sb")
                    e_v = e_sb[:].rearrange("p (t h) -> p t h", h=HEADS)
                    nc.vector.tensor_tensor(
                        out=e_v[:, 0:T1, :], in0=hgl_v[:, :, C1:D1],
                        in1=adg_v[:, 0:T1, :], op=mybir.AluOpType.add)
                    nc.vector.tensor_tensor(
                        out=e_v[:, T1:T, :], in0=hgh_v[:, :, C1:D1],
                        in1=adg_v[:, T1:T, :], op=mybir.AluOpType.add)
                    # leaky relu + exp
                    e2_sb = sm.tile([P, T * HEADS], fp32, tag="e2_sb")
                    nc.vector.tensor_scalar(out=e2_sb[:], in0=e_sb[:],
                                            scalar1=NEG_SLOPE, scalar2=None,
                                            op0=mybir.AluOpType.mult)
                    nc.vector.tensor_tensor(out=e2_sb[:], in0=e_sb[:], in1=e2_sb[:],
                                            op=mybir.AluOpType.max)
                    ex_sb = sm.tile([P, T * HEADS], fp32, tag="ex_sb")
                    nc.scalar.activation(out=ex_sb[:], in_=e2_sb[:],
                                         func=mybir.ActivationFunctionType.Exp)
                    # rhs = [h*ex | ex] per edge tile, fp16
                    rhs = wp.tile([P, T * D1], fp16, tag="rhs")
                    rhs_v = rhs[:].rearrange("p (t d) -> p t d", d=D1)
                    nc.scalar.copy(
                        out=rhs_v[:, :, C1:D1],
                        in_=ex_sb[:].rearrange("p (t h) -> p t h", h=HEADS))
                    nc.vector.tensor_tensor(
                        out=rhs_v[:, 0:T1, 0:C1].rearrange("p t (h c) -> p t h c", c=HID),
                        in0=hgl_v[:, :, 0:C1].rearrange("p t (h c) -> p t h c", c=HID),
                        in1=rhs_v[:, 0:T1, C1:D1].unsqueeze(3).to_broadcast([P, T1, HEADS, HID]),
                        op=mybir.AluOpType.mult)
                    nc.vector.tensor_tensor(
                        out=rhs_v[:, T1:T, 0:C1].rearrange("p t (h c) -> p t h c", c=HID),
                        in0=hgh_v[:, :, 0:C1].rearrange("p t (h c) -> p t h c", c=HID),
                        in1=rhs_v[:, T1:T, C1:D1].unsqueeze(3).to_broadcast([P, T2, HEADS, HID]),
                        op=mybir.AluOpType.mult)
                    # one-hot matmuls
                    ps1 = ps.tile([P, D1], fp32, tag="acc", space="PSUM")
                    for t in range(T):
                        eq = wp.tile([P, P], fp16, tag="eq")
                        nc.vector.tensor_scalar(
                            out=eq[:], in0=iota_sb[:],
                            scalar1=dstl_sb[:, jj * T + t:jj * T + t + 1],
                            scalar2=None, op0=mybir.AluOpType.is_equal)
                        nc.tensor.matmul(out=ps1[:], lhsT=eq[:],
                                         rhs=rhs[:, t * D1:(t + 1) * D1],
                                         start=(t == 0), stop=(t == T - 1))
                    # ---- post: h1 = ELU(mean_h(agg/den) + b1)
                    den = sm.tile([P, HEADS], fp32, tag="den")
                    nc.vector.tensor_scalar(out=den[:], in0=ps1[:, C1:D1],
                                            scalar1=1e-16, scalar2=None,
                                            op0=mybir.AluOpType.add)
                    rec = sm.tile([P, HEADS], fp32, tag="rec")
                    nc.vector.reciprocal(out=rec[:], in_=den[:])
                    tmp = sm.tile([P, C1], fp32, tag="tmp")
                    nc.vector.tensor_tensor(
                        out=tmp[:].rearrange("p (h c) -> p h c", c=HID),
                        in0=ps1[:, 0:C1].rearrange("p (h c) -> p h c", c=HID),
                        in1=rec[:].unsqueeze(2).to_broadcast([P, HEADS, HID]),
                        op=mybir.AluOpType.mult)
                    h1p = sm.tile([P, HID], fp32, tag="h1p")
                    nc.vector.tensor_reduce(
                        out=h1p[:], in_=tmp[:].rearrange("p (h c) -> p c h", c=HID),
                        axis=mybir.AxisListType.X, op=mybir.AluOpType.add)
                    h1b = sm.tile([P, HID], fp32, tag="h1b")
                    nc.vector.tensor_scalar(out=h1b[:], in0=h1p[:],
                                            scalar1=1.0 / HEADS, scalar2=None,
                                            op0=mybir.AluOpType.mult)
                    nc.vector.tensor_tensor(out=h1b[:], in0=h1b[:], in1=b1_sb[:],
                                            op=mybir.AluOpType.add)
                    xm = sm.tile([P, HID], fp32, tag="xm")
                    nc.vector.tensor_scalar(out=xm[:], in0=h1b[:], scalar1=0.0,
                                            scalar2=None, op0=mybir.AluOpType.min)
                    em = sm.tile([P, HID], fp32, tag="em")
                    nc.scalar.activation(out=em[:], in_=xm[:],
                                         func=mybir.ActivationFunctionType.Exp)
                    xp = sm.tile([P, HID], fp32, tag="xp")
                    nc.vector.tensor_scalar(out=xp[:], in0=h1b[:], scalar1=0.0,
                                            scalar2=None, op0=mybir.AluOpType.max)
                    h1 = sm.tile([P, HID], fp32, tag="h1")
                    nc.vector.tensor_tensor(out=h1[:], in0=em[:], in1=xp[:],
                                            op=mybir.AluOpType.add)
                    nc.vector.tensor_scalar(out=h1[:], in0=h1[:], scalar1=-1.0,
                                            scalar2=None, op0=mybir.AluOpType.add)
                    # ---- h2 row block
                    pst = ps2p.tile([HID, P], fp32, tag="mm2", space="PSUM")
                    nc.tensor.transpose(out=pst[:], in_=h1[:], identity=id_sb[:])
                    h1T = sm.tile([HID, P], fp16, tag="h1T")
                    nc.scalar.copy(out=h1T[:], in_=pst[:])
                    psh2 = ps2p.tile([P, N_CLS + 2], fp32, tag="mm2", space="PSUM")
                    nc.tensor.matmul(out=psh2[:], lhsT=h1T[:], rhs=w2_sb[:],
                                     start=True, stop=True)
                    h2st = sm.tile([P, N_CLS + 2], fp16, tag="h2st")
                    nc.scalar.copy(out=h2st[:], in_=psh2[:])
                    nc.sync.dma_start(out=h2_loc[jj * P:(jj + 1) * P, 0:N_CLS + 2],
                                      in_=h2st[:])
                    nc.sync.dma_start(out=h2_cmp_loc[jj * P:(jj + 1) * P, 0:N_CLS + 2],
                                      in_=h2st[:])
                jj_global += B

            # allgather compact h2 rows, then expand into the padded table
            nc.gpsimd.collective_compute(
                "AllGather", mybir.AluOpType.bypass, replica_groups=rg,
                ins=[h2_cmp_loc[:].opt()], outs=[h2_cmp_sh[:].opt()])
            nc.sync.dma_start(out=h2_ext[:, 0:N_CLS + 2],
                              in_=h2_cmp_sh[:, 0:N_CLS + 2])

            # ad2 pre-pass: gathers read only the device-local h2_loc, so they
            # run during the collective; keep just the a_d2 column.
            CH = 8
            col_d2 = 0
            for jj in range(TPD):
                ad2g = gp.tile([P, T * R2], fp16, tag="adg")
                for c0 in range(0, T, CH):
                    n_t = min(CH, T - c0); ni = n_t * P
                    nc.gpsimd.dma_gather(
                        out_ap=ad2g[:].rearrange("p (t d) -> p t d", d=R2)[:, c0:c0 + n_t, :],
                        in_ap=h2_loc[:, :],
                        idxs_ap=widx_sb[:, W_lo + W_hi + col_d2 + c0 * 8:W_lo + W_hi + col_d2 + (c0 + n_t) * 8],
                        num_idxs=ni, num_idxs_reg=ni, elem_size=R2)
                col_d2 += w_d[jj]
                nc.scalar.copy(
                    out=ad2c[:, jj * T:(jj + 1) * T].rearrange("p (t o) -> p t o", o=1),
                    in_=ad2g[:].rearrange("p (t d) -> p t d", d=R2)[:, :, N_CLS + 1:N_CLS + 2])

            # ---------------- conv2 main
            col_lo = col_hi = col_d = 0
            jj_global = 0
            CH = 8
            for g, B in enumerate(groups):
                hg2l = gp.tile([P, T1 * R2], fp16, tag="hg_lo")
                for c0 in range(0, T1, CH):
                    n_t = min(CH, T1 - c0); ni = n_t * P
                    nc.gpsimd.dma_gather(
                        out_ap=hg2l[:].rearrange("p (t d) -> p t d", d=R2)[:, c0:c0 + n_t, :],
                        in_ap=h2_ext[0:HALF, :],
                        idxs_ap=widx_sb[:, col_lo + c0 * 8:col_lo + (c0 + n_t) * 8],
                        num_idxs=ni, num_idxs_reg=ni, elem_size=R2)
                hg2h = gp.tile([P, T2 * R2], fp16, tag="hg_hi")
                for c0 in range(0, T2, CH):
                    n_t = min(CH, T2 - c0); ni = n_t * P
                    nc.gpsimd.dma_gather(
                        out_ap=hg2h[:].rearrange("p (t d) -> p t d", d=R2)[:, c0:c0 + n_t, :],
                        in_ap=h2_ext[HALF:N_PAD, :],
                        idxs_ap=widx_sb[:, W_lo + col_hi + c0 * 8:W_lo + col_hi + (c0 + n_t) * 8],
                        num_idxs=ni, num_idxs_reg=ni, elem_size=R2)
                col_lo += w_lo[g]; col_hi += w_hi[g]; col_d += w_d[g]

                for k in range(B):
                    jj = jj_global + k
                    h2l_v = hg2l[:].rearrange("p (t d) -> p t d", d=R2)[:, k * T1:(k + 1) * T1, :]
                    h2h_v = hg2h[:].rearrange("p (t d) -> p t d", d=R2)[:, k * T2:(k + 1) * T2, :]
                    ad2_v = ad2c[:, jj * T:(jj + 1) * T].rearrange("p (t o) -> p t o", o=1)
                    e2s = sm.tile([P, T], fp32, tag="e2s")
                    e2v = e2s[:].rearrange("p (t o) -> p t o", o=1)
                    nc.vector.tensor_tensor(
                        out=e2v[:, 0:T1, :], in0=h2l_v[:, :, N_CLS:N_CLS + 1],
                        in1=ad2_v[:, 0:T1, :], op=mybir.AluOpType.add)
                    nc.vector.tensor_tensor(
                        out=e2v[:, T1:T, :], in0=h2h_v[:, :, N_CLS:N_CLS + 1],
                        in1=ad2_v[:, T1:T, :], op=mybir.AluOpType.add)
                    e2m = sm.tile([P, T], fp32, tag="e2m")
                    nc.vector.tensor_scalar(out=e2m[:], in0=e2s[:],
                                            scalar1=NEG_SLOPE, scalar2=None,
                                            op0=mybir.AluOpType.mult)
                    nc.vector.tensor_tensor(out=e2m[:], in0=e2s[:], in1=e2m[:],
                                            op=mybir.AluOpType.max)
                    ex2 = sm.tile([P, T], fp32, tag="ex2")
                    nc.scalar.activation(out=ex2[:], in_=e2m[:],
                                         func=mybir.ActivationFunctionType.Exp)
                    D2 = N_CLS + 1
                    rhs2 = wp.tile([P, T * D2], fp16, tag="rhs")
                    rhs2_v = rhs2[:].rearrange("p (t d) -> p t d", d=D2)
                    nc.scalar.copy(
                        out=rhs2_v[:, :, N_CLS:D2],
                        in_=ex2[:].rearrange("p (t o) -> p t o", o=1))
                    nc.vector.tensor_tensor(
                        out=rhs2_v[:, 0:T1, 0:N_CLS],
                        in0=h2l_v[:, :, 0:N_CLS],
                        in1=rhs2_v[:, 0:T1, N_CLS:D2].to_broadcast([P, T1, N_CLS]),
                        op=mybir.AluOpType.mult)
                    nc.vector.tensor_tensor(
                        out=rhs2_v[:, T1:T, 0:N_CLS],
                        in0=h2h_v[:, :, 0:N_CLS],
                        in1=rhs2_v[:, T1:T, N_CLS:D2].to_broadcast([P, T2, N_CLS]),
                        op=mybir.AluOpType.mult)
                    ps3 = ps.tile([P, D2], fp32, tag="acc", space="PSUM")
                    for t in range(T):
                        eq = wp.tile([P, P], fp16, tag="eq")
                        nc.vector.tensor_scalar(
                            out=eq[:], in0=iota_sb[:],
                            scalar1=dstl_sb[:, jj * T + t:jj * T + t + 1],
                            scalar2=None, op0=mybir.AluOpType.is_equal)
                        nc.tensor.matmul(out=ps3[:], lhsT=eq[:],
                                         rhs=rhs2[:, t * D2:(t + 1) * D2],
                                         start=(t == 0), stop=(t == T - 1))
                    # ---- post: log_softmax(agg/den + b2)
                    d2n = sm.tile([P, 1], fp32, tag="d2n")
                    nc.vector.tensor_scalar(out=d2n[:], in0=ps3[:, N_CLS:D2],
                                            scalar1=1e-16, scalar2=None,
                                            op0=mybir.AluOpType.add)
                    r2 = sm.tile([P, 1], fp32, tag="r2")
                    nc.vector.reciprocal(out=r2[:], in_=d2n[:])
                    h2f = sm.tile([P, N_CLS], fp32, tag="h2f")
                    nc.vector.tensor_tensor(out=h2f[:], in0=ps3[:, 0:N_CLS],
                                            in1=r2[:].to_broadcast([P, N_CLS]),
                                            op=mybir.AluOpType.mult)
                    nc.vector.tensor_tensor(out=h2f[:], in0=h2f[:], in1=b2_sb[:],
                                            op=mybir.AluOpType.add)
                    nm = sm.tile([P, 1], fp32, tag="nm")
                    nc.vector.tensor_reduce(out=nm[:], in_=h2f[:],
                                            axis=mybir.AxisListType.X,
                                            op=mybir.AluOpType.max, negate=True)
                    es = sm.tile([P, N_CLS], fp32, tag="es")
                    nc.scalar.activation(out=es[:], in_=h2f[:],
                                         func=mybir.ActivationFunctionType.Exp,
                                         bias=nm[:])
                    ssum = sm.tile([P, 1], fp32, tag="ssum")
                    nc.vector.tensor_reduce(out=ssum[:], in_=es[:],
                                            axis=mybir.AxisListType.X,
                                            op=mybir.AluOpType.add)
                    yi = sm.tile([P, 1], mybir.dt.int32, tag="yi")
                    nc.vector.tensor_scalar(out=yi[:], in0=ssum[:].bitcast(mybir.dt.int32),
                                            scalar1=1065353216, scalar2=None,
                                            op0=mybir.AluOpType.subtract)
                    yf = sm.tile([P, 1], fp32, tag="yf")
                    nc.vector.tensor_copy(out=yf[:], in_=yi[:])
                    lg = sm.tile([P, 1], fp32, tag="lg")
                    nc.vector.tensor_scalar(out=lg[:], in0=yf[:],
                                            scalar1=8.262958405176314e-08,
                                            scalar2=None, op0=mybir.AluOpType.mult)
                    for _ in range(2):
                        eny = sm.tile([P, 1], fp32, tag="eny")
                        nc.scalar.activation(out=eny[:], in_=lg[:],
                                             func=mybir.ActivationFunctionType.Exp,
                                             scale=-1.0)
                        u_ = sm.tile([P, 1], fp32, tag="u_")
                        nc.vector.tensor_tensor(out=u_[:], in0=eny[:], in1=ssum[:],
                                                op=mybir.AluOpType.mult)
                        nc.vector.tensor_tensor(out=lg[:], in0=lg[:], in1=u_[:],
                                                op=mybir.AluOpType.add)
                        nc.vector.tensor_scalar(out=lg[:], in0=lg[:], scalar1=-1.0,
                                                scalar2=None, op0=mybir.AluOpType.add)
                    sh = sm.tile([P, 1], fp32, tag="sh")
                    nc.vector.tensor_tensor(out=sh[:], in0=nm[:], in1=lg[:],
                                            op=mybir.AluOpType.subtract)
                    ot = sm.tile([P, N_CLS], fp32, tag="ot")
                    nc.vector.tensor_scalar(out=ot[:], in0=h2f[:], scalar1=sh[:],
                                            scalar2=None, op0=mybir.AluOpType.add)
                    nc.sync.dma_start(out=out[jj * P:(jj + 1) * P, :], in_=ot[:])
                jj_global += B

    nc.compile()
    return nc


def _make_in_maps(inputs, dev_widx, dev_dstl, cfg):
    x = np.asarray(inputs["x"], dtype=np.float32)
    W1 = np.asarray(inputs["W1"], dtype=np.float32)
    att_src1 = np.asarray(inputs["att_src1"], dtype=np.float32)
    att_dst1 = np.asarray(inputs["att_dst1"], dtype=np.float32)
    b1 = np.asarray(inputs["b1"], dtype=np.float32)
    W2 = np.asarray(inputs["W2"], dtype=np.float32)
    att_src2 = np.asarray(inputs["att_src2"], dtype=np.float32)
    att_dst2 = np.asarray(inputs["att_dst2"], dtype=np.float32)
    b2 = np.asarray(inputs["b2"], dtype=np.float32)

    As = np.zeros((C1, HEADS), np.float32)
    Ad = np.zeros((C1, HEADS), np.float32)
    for h in range(HEADS):
        As[h * HID:(h + 1) * HID, h] = att_src1[h]
        Ad[h * HID:(h + 1) * HID, h] = att_dst1[h]
    w1ext = np.concatenate([W1, W1 @ As, W1 @ Ad], axis=1).astype(np.float16)
    w2ext = np.concatenate(
        [W2, (W2 @ att_src2[0])[:, None], (W2 @ att_dst2[0])[:, None]],
        axis=1).astype(np.float16)

    x_pad = np.zeros((N_PAD, F_IN), np.float32)
    x_pad[:N_NODES] = x
    xT = np.ascontiguousarray(x_pad.T.astype(np.float16))

    b1b = np.tile(b1[None, :], (P, 1)).astype(np.float32)
    b2b = np.tile(b2[None, :], (P, 1)).astype(np.float32)
    ident = np.eye(P, dtype=np.float32)
    iota = np.tile(np.arange(P, dtype=np.float16)[None, :], (P, 1))

    in_maps = []
    for d in range(N_DEV):
        in_maps.append({
            "xT": xT, "w1ext": w1ext, "w2ext": w2ext, "b1b": b1b, "b2b": b2b,
            "ident": ident, "iota": np.ascontiguousarray(iota),
            "widx": dev_widx[d], "dstli": dev_dstl[d],
            "xT_loc": np.ascontiguousarray(xT[:, d * NPD:(d + 1) * NPD]),
        })
    return in_maps


def kernel(x, edge_index, W1, att_src1, att_dst1, b1, W2, att_src2, att_dst2, b2):
    edge_index = np.asarray(edge_index, dtype=np.int32)
    dev_widx, dev_dstl, cfg = _preprocess(edge_index)

    key = (cfg["T1"], cfg["T2"], tuple(cfg["groups"]))
    if key not in _CACHE:
        _CACHE[key] = _build(cfg)
    nc = _CACHE[key]

    in_maps = _make_in_maps(dict(
        x=x, W1=W1, att_src1=att_src1, att_dst1=att_dst1, b1=b1,
        W2=W2, att_src2=att_src2, att_dst2=att_dst2, b2=b2),
        dev_widx, dev_dstl, cfg)
    res = run_bass_kernel_spmd(nc, in_maps, list(range(N_DEV)))
    full = np.concatenate([res.results[d]["out"] for d in range(N_DEV)], axis=0)
    return full[:N_NODES]
